# revision 2
# baseline (speedup 1.0000x reference)
"""Trainium2 Bass kernel for gated pair-bias attention (B=8,S=1024,D=256,H=8,DH=32).

Sharding: data-parallel over batch — core b computes batch element b entirely;
weights + pair bias replicated to all 8 cores.

Differences vs v1 (381us baseline):
  - bias add no longer rides PE identity matmuls. Host precomputes
    EB = exp(bias^T + mask) in bf16; on-chip the softmax numerator is
    attn = exp(scores) * EB, a 2x-mode DVE multiply. Removes 128 N=512
    matmuls (~27us PE) and the PE->PE serialization they caused.
  - EB is DMA'd as 16 x 1MB transfers on the gpsimd SWDGE ring (Pool engine
    is otherwise idle), not 64 x 256KB on the SP HWDGE ring. 1MB transfers
    run at ~341GB/s vs ~240 for 256KB, and the SP ring keeps qkv/weights/out
    unqueued behind bias bytes.
  - epilogue (gate * o / sigma) operates on the attnV psum pair-layout
    directly with two-block strided partition APs ({0-31,64-95} o rows,
    {32-63,96-127} sigma rows), with the output projection reading the
    pair-layout tiles against a host-permuted WoT whose sigma rows are 0.
    DVE epilogue drops ~28us -> ~11us.
  - gate tiles kept in bf16 (4x-mode copies).

Per-core math (batch index dropped):
  g     = sigmoid(q @ Wg^T + bg)                      [S, E]
  qh    = (q @ Wq^T) * DH^-0.5 ; kh = k @ Wk^T ; vh = v @ Wv^T
  attn  = exp(qh_h @ kh_h^T) * EB_h                   (flash-free: |scores|<1)
  o     = attn @ vh_h ; sigma = attn @ 1 ;  o = g * o / sigma ; out = o @ Wo^T
"""

import os
import sys

import numpy as np

for _p in ("/opt/trn_rl_repo", "/root/.axon_site/_ro/trn_rl_repo"):
    if os.path.isdir(_p) and _p not in sys.path:
        sys.path.append(_p)

import ml_dtypes
import concourse.bass as bass
import concourse.mybir as mybir
import concourse.tile as tile
from concourse import bacc
from concourse.bass_utils import run_bass_kernel_spmd

S, D, E, H, DH = 1024, 256, 256, 8, 32
NCORES = 8
F32 = mybir.dt.float32
BF16 = mybir.dt.bfloat16
NORM = float(DH) ** -0.5
ST = S // 128   # 8 s-tiles
DT = D // 128   # 2 d-tiles
ET = E // 128   # 2 e-tiles
Act = mybir.ActivationFunctionType


def build_bass(repeat: int = 1, bias_internal: bool = False) -> bass.Bass:
    # Bacc (not raw Bass): its compile() runs move_matmul_waits_to_ldweights +
    # generate_event_semaphores, which split multi-semaphore waits that the
    # TRN2 instruction encodings cannot carry.
    nc = bacc.Bacc("TRN2", target_bir_lowering=False, debug=True)

    qT_d = nc.dram_tensor("qT", [D, S], BF16, kind="ExternalInput")
    kT_d = nc.dram_tensor("kT", [D, S], BF16, kind="ExternalInput")
    vT_d = nc.dram_tensor("vT", [D, S], BF16, kind="ExternalInput")
    # EB pre-tiled on host: [H, 2 halves, 128, 4096]; tile (h, half) holds
    # bias^T rows half*512..+512 as sbuf (p, a*1024+q) = ebT[h, half*512+
    # a*128+p, q] so each DMA is 128 partitions x 8KB contiguous.
    if bias_internal:  # timing-only variant: garbage EB, no 17MB upload
        ebT_d = nc.dram_tensor("EBT", [H, 2, 128, 4096], BF16)
    else:
        ebT_d = nc.dram_tensor("EBT", [H, 2, 128, 4096], BF16, kind="ExternalInput")
    w_d = {  # all pre-transposed on host; "q" also pre-scaled by DH^-0.5
        "q": nc.dram_tensor("WqT", [D, E], BF16, kind="ExternalInput"),
        "k": nc.dram_tensor("WkT", [D, E], BF16, kind="ExternalInput"),
        "v": nc.dram_tensor("WvT", [D, E], BF16, kind="ExternalInput"),
        "g": nc.dram_tensor("WgT", [D, E], BF16, kind="ExternalInput"),
    }
    # pair-layout output weights: [4 pairs, 128, D]; sigma rows zeroed
    woP_d = nc.dram_tensor("WoP", [4, 128, D], BF16, kind="ExternalInput")
    bg_d = nc.dram_tensor("bg", [E], F32, kind="ExternalInput")
    # bf16 output: harness tolerance is 2e-2 rel; bf16 adds ~0.4%. Halves
    # the store bytes on the kernel's tail.
    out_d = nc.dram_tensor("out", [S, D], BF16, kind="ExternalOutput")

    with tile.TileContext(nc) as tc:
        with (
            tc.tile_pool(name="const", bufs=1) as constp,
            tc.tile_pool(name="persist", bufs=1) as persist,
            tc.tile_pool(name="ebp", bufs=8) as ebp,
            tc.tile_pool(name="expp", bufs=4) as expp,
            tc.tile_pool(name="attp", bufs=4) as attp,
            tc.tile_pool(name="smallp", bufs=4) as smallp,
            tc.tile_pool(name="outp", bufs=3) as outp,
            tc.tile_pool(name="psum", bufs=2, space="PSUM") as psum,
        ):
            # ALL DMAs ride the SP HWDGE ring in need-order (small loads
            # first, EB tiles after, output stores last). A DMA issue
            # occupies the issuing SEQ until the transfer starts, so any
            # ring on a compute engine would stall that engine's stream
            # behind queued transfers; SP has nothing else to do.
            kT, qT, vT = [], [], []
            WT = {nm: [] for nm in ("k", "q", "g", "v")}

            def load_w(nm):
                for i in range(DT):
                    wt = constp.tile([128, E], BF16, name=f"WT_{nm}{i}",
                                     tag=f"WT_{nm}{i}")
                    nc.sync.dma_start(out=wt[:],
                                      in_=w_d[nm][i * 128 : (i + 1) * 128, :])
                    WT[nm].append(wt)

            for pref, dst, src_d in (("k", kT, kT_d), ("q", qT, qT_d)):
                for i in range(DT):
                    t = persist.tile([128, S], BF16, name=f"{pref}T{i}",
                                     tag=f"{pref}T{i}")
                    nc.sync.dma_start(out=t[:], in_=src_d[i * 128 : (i + 1) * 128, :])
                    dst.append(t)
                load_w(pref)  # weight right behind its operand: kh proj
                #              starts after 4 transfers, not 8
            load_w("g")
            load_w("v")
            bg_sb = constp.tile([128, ET], F32)
            bg2d = bg_d.rearrange("(a b) -> a b", b=1)
            for et in range(ET):
                nc.sync.dma_start(out=bg_sb[:, et : et + 1],
                                    in_=bg2d[et * 128 : (et + 1) * 128, :])
            for i in range(DT):
                t = persist.tile([128, S], BF16, name=f"vT{i}", tag=f"vT{i}")
                nc.sync.dma_start(out=t[:], in_=vT_d[i * 128 : (i + 1) * 128, :])
                vT.append(t)
            woP = []
            for j in range(4):
                wp = constp.tile([128, D], BF16, name=f"WoP{j}", tag=f"WoP{j}")
                nc.sync.dma_start(out=wp[:], in_=woP_d[j])
                woP.append(wp)

            o_gP = [persist.tile([128, S], BF16, name=f"o_gP{j}") for j in range(4)]

            for _rep in range(repeat):
                # ---- EB prefetch: 1MB HWDGE transfers ----
                # EB half-tile (h, half) = [128, 4096]: element (k=a*128+p, q)
                # of biasT head h rows half*512.. -> sbuf (p, a*1024+q).
                # Pairs 0-1 ride the scalar ring FIFO right behind q/k/v, in
                # half-major order (both heads' first halves before second
                # halves). Pairs 2-3 ride the otherwise-idle SP ring,
                # staggered one pair ahead of use — their ebp buffer waits
                # would deadlock any ring whose SEQ later issues compute.
                eb = {}

                def issue_eb_pair(p, eng):
                    for half in range(2):
                        for h in (2 * p, 2 * p + 1):
                            t = ebp.tile([128, 4096], BF16, tag="eb",
                                         name=f"eb_h{h}_{half}")
                            eng.dma_start(out=t[:], in_=ebT_d[h, half])
                            eb[(h, half)] = t

                issue_eb_pair(0, nc.sync)

                # o_gP sigma rows ({32-63, 96-127}) must be finite-and-zero:
                # the output projection contracts all 128 partitions against
                # WoP (whose sigma rows are zero too). On gpsimd, after the
                # EB DMA issues: runs in the Pool engine's shadow, long
                # before the output projection needs it.
                for j in range(4):
                    for ro in (DH, 64 + DH):
                        nc.gpsimd.memset(o_gP[j][ro : ro + DH, :], 0.0)

                # ---- projections ----
                # vh_aug ones columns first: DVE is idle before the proj
                # evacuations, and vh_aug[st] must be complete before attnV.
                vh_aug = [persist.tile([128, 8 * 64], BF16, name=f"vh_aug{i}")
                          for i in range(ST)]
                for st in range(ST):
                    nc.vector.memset(
                        vh_aug[st].rearrange("p (h c) -> p h c", c=64)[:, :, DH : 2 * DH],
                        1.0)

                khT = [persist.tile([128, S], BF16, name=f"khT{i}") for i in range(ET)]
                qhT = [persist.tile([128, S], BF16, name=f"qhT{i}") for i in range(ET)]
                gateT = [persist.tile([128, S], BF16, name=f"gateT{i}") for i in range(ET)]

                def proj_et(dst_cb, wname, xT, et, tag="ps_big", bufs=3):
                    ps_p = psum.tile([128, S], F32, tag=tag, bufs=bufs,
                                     name=f"ps_{wname}{et}")
                    for dt in range(DT):
                        for qc in range(2):
                            nc.tensor.matmul(
                                ps_p[:, qc * 512 : (qc + 1) * 512],
                                lhsT=WT[wname][dt][:, et * 128 : (et + 1) * 128],
                                rhs=xT[dt][:, qc * 512 : (qc + 1) * 512],
                                start=(dt == 0), stop=(dt == DT - 1))
                    dst_cb(ps_p)

                def proj_kq(et):
                    proj_et(lambda ps: nc.vector.tensor_copy(khT[et][:], ps[:]),
                            "k", kT, et)
                    proj_et(lambda ps: nc.vector.tensor_copy(qhT[et][:], ps[:]),
                            "q", qT, et)

                # et0 (heads 0-3, pairs 0-1) now; et1 deferred into pair 1.
                proj_kq(0)

                # gateT in bf16; read only at pair epilogues. Projected through
                # the ps_o bank (ps_big stays free for pair-0 scores), BEFORE
                # the vh projection so ACT's sigmoid (+ its table load)
                # happens early and the exp stream starts sooner.
                for et in range(ET):
                    ps_g = psum.tile([128, S], F32, tag="ps_o", bufs=1, name="ps_g")
                    for dt in range(DT):
                        for qc in range(2):
                            nc.tensor.matmul(
                                ps_g[:, qc * 512 : (qc + 1) * 512],
                                lhsT=WT["g"][dt][:, et * 128 : (et + 1) * 128],
                                rhs=qT[dt][:, qc * 512 : (qc + 1) * 512],
                                start=(dt == 0), stop=(dt == DT - 1))
                    nc.scalar.activation(gateT[et][:], ps_g[:], Act.Sigmoid,
                                         bias=bg_sb[:, et : et + 1])

                for st in range(ST):
                    ps_v = psum.tile([128, E], F32, tag="ps_big", bufs=3, name="ps_v")
                    for dt in range(DT):
                        nc.tensor.matmul(ps_v[:],
                                         lhsT=vT[dt][:, st * 128 : (st + 1) * 128],
                                         rhs=WT["v"][dt][:],
                                         start=(dt == 0), stop=(dt == DT - 1))
                    nc.vector.tensor_copy(
                        vh_aug[st].rearrange("p (h c) -> p h c", c=64)[:, :, 0:DH],
                        ps_v[:].rearrange("p (h c) -> p h c", c=DH))

                # ---- attention, head PAIRS (2j, 2j+1) ----
                # Pair scores use two 32-row PE groups (tile_position) and the
                # pair's attn@V two 64-col groups -> concurrent on HW.
                gate_pair = [persist.tile([128, S], BF16, name=f"gate_pair{j}")
                             for j in range(4)]

                # epilogue of pair j (one 64-row half per call): gate copy,
                # 1/sigma, gate*o, *1/sigma. Software-pipelined into pair
                # j+1's kt=0/kt=1 slots so the DVE stream keeps feeding
                # ACT/PE; pair j+1's attnV kt=0 absorbs the ps_o WAR wait.
                def epilogue_half(j, ps_o, ro, cs=slice(0, S)):
                    et = (2 * j) // 4
                    hr = ((2 * j + ro // 64) % 4) * DH
                    nc.vector.tensor_copy(gate_pair[j][ro : ro + DH, cs],
                                          gateT[et][hr : hr + DH, cs])
                    rsig = smallp.tile([128, S], BF16, tag="rsig")
                    tgo = smallp.tile([128, S], BF16, tag="tgo")
                    with nc.allow_low_precision(
                            reason="1/sigma in bf16: 0.4% rel, checked vs ref"):
                        nc.vector.reciprocal(rsig[ro : ro + DH, cs],
                                             ps_o[ro + DH : ro + 2 * DH, cs])
                    nc.vector.tensor_mul(tgo[ro : ro + DH, cs],
                                         ps_o[ro : ro + DH, cs],
                                         gate_pair[j][ro : ro + DH, cs])
                    nc.vector.tensor_mul(o_gP[j][ro : ro + DH, cs],
                                         tgo[ro : ro + DH, cs],
                                         rsig[ro : ro + DH, cs])

                def attnV(kt, atts, ps_o, hA, hB):
                    for qc in range(2):
                        qcs = slice(qc * 512, (qc + 1) * 512)
                        nc.tensor.matmul(
                            ps_o[0:64, qcs],
                            lhsT=vh_aug[kt][:, hA * 64 : (hA + 1) * 64],
                            rhs=atts[hA][:, qcs],
                            start=(kt == 0), stop=(kt == ST - 1),
                            tile_position=(0, 0))
                        nc.tensor.matmul(
                            ps_o[64:128, qcs],
                            lhsT=vh_aug[kt][:, hB * 64 : (hB + 1) * 64],
                            rhs=atts[hB][:, qcs],
                            start=(kt == 0), stop=(kt == ST - 1),
                            tile_position=(0, 64))

                prev = None  # (j, ps_o) awaiting epilogue
                pend = None  # attnV args of kt-1, emitted after scores(kt),
                #              carried across pair boundaries
                for j in range(H // 2):
                    hA, hB = 2 * j, 2 * j + 1
                    et = hA // 4
                    hrA, hrB = (hA % 4) * DH, (hB % 4) * DH
                    if j <= 2:
                        issue_eb_pair(j + 1, nc.sync)
                    ps_o = psum.tile([128, S], F32, tag="ps_o", bufs=1)
                    for kt in range(ST):
                        ps_s = {
                            hA: psum.tile([128, S], F32, tag="ps_big", bufs=3,
                                          name="ps_sA"),
                            hB: psum.tile([128, S], F32, tag="ps_big", bufs=3,
                                          name="ps_sB"),
                        }
                        for qc in range(2):
                            for hh, hr in ((hA, hrA), (hB, hrB)):
                                nc.tensor.matmul(
                                    ps_s[hh][:, qc * 512 : (qc + 1) * 512],
                                    lhsT=khT[et][hr : hr + DH,
                                                 kt * 128 : (kt + 1) * 128],
                                    rhs=qhT[et][hr : hr + DH,
                                                qc * 512 : (qc + 1) * 512],
                                    start=True, stop=True,
                                    tile_position=(hr, 0))
                        # ordering around the reused ps_o (bufs=1): at kt=0
                        # flush attnV(j-1,kt7) (last old-gen writer) before
                        # epilogue half 0 (old-gen reader); at kt=1 emit half
                        # 1 BEFORE flushing attnV(j,kt0) (new-gen writer).
                        if kt == 0 and pend is not None:
                            attnV(*pend)
                            pend = None
                        if prev is not None and kt < 2:
                            epilogue_half(*prev, 64 * kt)
                            if kt == 1:
                                prev = None
                        if kt != 0 and pend is not None:
                            attnV(*pend)
                            pend = None
                        atts = {}
                        for hh in (hA, hB):
                            expT = expp.tile([128, S], BF16, tag="expT",
                                             name=f"expT{hh % 2}")
                            nc.scalar.activation(expT[:], ps_s[hh][:], Act.Exp)
                            att = attp.tile([128, S], BF16, tag="att",
                                            name=f"att{hh % 2}")
                            ebt = eb[(hh, kt // 4)]
                            ebs = ebt[:, (kt % 4) * S : (kt % 4 + 1) * S]
                            nc.vector.tensor_mul(att[:], expT[:], ebs)
                            atts[hh] = att
                        if j == 1 and kt == 5:
                            proj_kq(1)  # heads 4-7, needed from pair 2
                        pend = (kt, atts, ps_o, hA, hB)
                    prev = (j, ps_o)
                attnV(*pend)
                # final pair's epilogue in 256-col chunks: the output
                # projection's first s-tiles depend only on the first chunk.
                for cq in range(2):
                    for ro in (0, 64):
                        epilogue_half(*prev, ro, slice(cq * 512, (cq + 1) * 512))

                # ---- output projection: contract pair-layout tiles vs WoP --
                # j=0..2 immediately (their o_gP are long done); j=3 deferred
                # two s-tiles so PE has filler while pair 3's epilogue drains.
                ps_outs = {}

                def fin_out(st):
                    ps_out = ps_outs.pop(st)
                    nc.tensor.matmul(ps_out[:],
                                     lhsT=o_gP[3][:, st * 128 : (st + 1) * 128],
                                     rhs=woP[3][:], start=False, stop=True)
                    o_sb = outp.tile([128, D], BF16, tag="o_sb")
                    nc.vector.tensor_copy(o_sb[:], ps_out[:])
                    nc.sync.dma_start(out=out_d[st * 128 : (st + 1) * 128, :],
                                      in_=o_sb[:])

                for st in range(ST):
                    ps_out = psum.tile([128, D], F32, tag="ps_big", bufs=3,
                                       name="ps_out")
                    for j in range(3):
                        nc.tensor.matmul(ps_out[:],
                                         lhsT=o_gP[j][:, st * 128 : (st + 1) * 128],
                                         rhs=woP[j][:],
                                         start=(j == 0), stop=False)
                    ps_outs[st] = ps_out
                    if st >= 2:
                        fin_out(st - 2)
                fin_out(ST - 2)
                fin_out(ST - 1)

    nc.compile()
    return nc


_CACHED = {}


def run(inputs: dict, trace: bool = False, **spmd_kwargs):
    if "nc" not in _CACHED:
        _CACHED["nc"] = build_bass()
    nc = _CACHED["nc"]

    f32 = np.float32
    bf16 = ml_dtypes.bfloat16
    q = np.asarray(inputs["q"], dtype=f32)
    k = np.asarray(inputs["k"], dtype=f32)
    v = np.asarray(inputs["v"], dtype=f32)
    mask = np.asarray(inputs["mask"], dtype=f32)
    bias = np.asarray(inputs["bias"], dtype=f32).reshape(H, S, S)

    wqT = np.ascontiguousarray((np.asarray(inputs["Wq"], dtype=f32).T * NORM).astype(bf16))
    wkT = np.ascontiguousarray(np.asarray(inputs["Wk"], dtype=f32).T.astype(bf16))
    wvT = np.ascontiguousarray(np.asarray(inputs["Wv"], dtype=f32).T.astype(bf16))
    wgT = np.ascontiguousarray(np.asarray(inputs["Wg"], dtype=f32).T.astype(bf16))
    bg = np.ascontiguousarray(np.asarray(inputs["bg"], dtype=f32))

    # pair-layout Wo: pair j rows 0-31 = head 2j block, rows 64-95 = head 2j+1
    # block, sigma rows zero.
    woT = np.asarray(inputs["Wo"], dtype=f32).T  # [E, D]
    woP = np.zeros((4, 128, D), f32)
    for j in range(4):
        woP[j, 0:DH] = woT[64 * j : 64 * j + DH]
        woP[j, 64 : 64 + DH] = woT[64 * j + DH : 64 * j + 2 * DH]
    woP = np.ascontiguousarray(woP.astype(bf16))

    # EBT[h, k, q] = exp(bias[h, q, k] + mask[b, k]), pre-tiled to
    # [H, 2, 128, 4096] so each sbuf tile is one contiguous DMA:
    # (h, half, p, a*1024+q) = ebT[h, half*512 + a*128 + p, q].
    def tile_eb(ebT):
        return np.ascontiguousarray(
            ebT.reshape(H, 2, 4, 128, S).transpose(0, 1, 3, 2, 4)
            .reshape(H, 2, 128, 4096).astype(bf16))

    ebT_shared = tile_eb(np.exp(bias.transpose(0, 2, 1)))

    B = q.shape[0]
    in_maps = []
    for b in range(B):
        if np.any(mask[b]):
            ebT_b = tile_eb(
                np.exp(bias.transpose(0, 2, 1) + mask[b].reshape(1, S, 1)))
        else:
            ebT_b = ebT_shared
        in_maps.append({
            "qT": np.ascontiguousarray(q[b].T.astype(bf16)),
            "kT": np.ascontiguousarray(k[b].T.astype(bf16)),
            "vT": np.ascontiguousarray(v[b].T.astype(bf16)),
            "EBT": ebT_b,
            "WqT": wqT, "WkT": wkT, "WvT": wvT, "WgT": wgT, "WoP": woP,
            "bg": bg,
        })
    res = run_bass_kernel_spmd(nc, in_maps, list(range(NCORES)),
                               trace=trace, **spmd_kwargs)
    out = np.stack([res.results[i]["out"] for i in range(NCORES)],
                   axis=0).astype(np.float32)
    return out, res


def kernel(**inputs) -> np.ndarray:
    out, _ = run(inputs)
    return out.astype(np.float32)
